# revision 15
# baseline (speedup 1.0000x reference)
"""Trainium2 Bass kernel for nn_KLFocalLossColBERT.

Reference computation (B=128, LQ=32, LD=256, D=128, NWAY=16, GAMMA=5):
  q  = l2norm(query_reps, axis=2)                     # over D
  d  = l2norm(doc_reps * doc_masks[..., None], axis=2)  # over Ld (token axis)
  sim = einsum('bqd,nbld->nbql', q, d)
  scores[b, n] = sum_q max_l sim
  logp = log_softmax(scores, -1); p = exp(logp); t = labels[:, :NWAY]
  loss = mean(exp(t) * (t - logp) * p**GAMMA)

Sharding: data-parallel over batch B across 8 cores (16 examples each).
Each core returns a [1,1] partial sum of loss entries; host sums / (B*NWAY).

Per-core pipeline per (b, n) pair:
  - DMA doc[n,b] [256,128] into SBUF as [128p, 2c, 128d] (l = c*128+p)
  - mask via per-partition tensor_scalar (maskT pre-transposed once on PE)
  - 2x PE transpose -> PSUM dT [128d, 256l]
  - ACT copy PSUM->SBUF
  - DVE tensor_tensor_reduce -> sumsq over l per feature d
  - rsqrt folded into the small qT operand (not the big doc tile)
  - PE matmul sim = qT_scaled.T @ dT  -> PSUM [32, 256]
  - DVE reduce_max -> maxsim staging column
Tail: ones-matmul -> scores, softmax/KL/focal on a [16,16] tile.
"""

import os
import sys

import numpy as np

for _p in ("/opt/trn_rl_repo", "/root/.axon_site/_ro/trn_rl_repo"):
    if os.path.isdir(_p) and _p not in sys.path:
        sys.path.insert(0, _p)

import concourse.bass as bass
import concourse.bacc as bacc_mod
import concourse.mybir as mybir
from concourse import bass_utils
from concourse.masks import make_identity
from concourse.tile import TileContext

F32 = mybir.dt.float32
I32 = mybir.dt.int32
AF = mybir.ActivationFunctionType
ALU = mybir.AluOpType

B, LQ, LD, D, NWAY = 128, 32, 256, 128, 16
GAMMA = 5
NCORES = 8
BL = B // NCORES  # 16 local examples per core

_nc_cache = None


def _build_nc():
    nc = bacc_mod.Bacc()
    q_d = nc.dram_tensor("q", [BL, LQ, D], F32, kind="ExternalInput")
    doc_d = nc.dram_tensor("doc", [NWAY, BL, LD, D], F32, kind="ExternalInput")
    msk_d = nc.dram_tensor("msk", [NWAY, BL, LD], I32, kind="ExternalInput")
    lab_d = nc.dram_tensor("lab", [BL, 2 * NWAY], F32, kind="ExternalInput")
    out_d = nc.dram_tensor("out", [1, 1], F32, kind="ExternalOutput")
    q_ap, doc_ap, msk_ap, lab_ap, out_ap = (
        q_d[:], doc_d[:], msk_d[:], lab_d[:], out_d[:]
    )

    with TileContext(nc) as tc:
        with (
            tc.tile_pool(name="consts", bufs=1) as consts,
            tc.tile_pool(name="apool", bufs=6) as apool,
            tc.tile_pool(name="rpool", bufs=20) as rpool,
            tc.tile_pool(name="scratch", bufs=2) as scratch,
            tc.tile_pool(name="small", bufs=4) as small,
            tc.tile_pool(name="qpool", bufs=3) as qpool,
            tc.tile_pool(name="ps_dt", bufs=3, space="PSUM") as ps_dt,
            tc.tile_pool(name="ps_sim", bufs=3, space="PSUM") as ps_sim,
            tc.tile_pool(name="ps_misc", bufs=2, space="PSUM") as ps_misc,
            tc.tile_pool(name="dram", bufs=1, space="DRAM") as dram,
        ):
            ident_g = consts.tile([128, 128], F32, tag="ident_g")
            make_identity(nc, ident_g)
            # re-materialize via DVE so PE matmuls wait on a single engine
            ident = consts.tile([128, 128], F32, tag="ident")
            nc.vector.tensor_copy(ident, ident_g)
            ones32 = consts.tile([32, 1], F32)
            nc.vector.memset(ones32, 1.0)
            ones16 = consts.tile([16, 1], F32)
            nc.vector.memset(ones16, 1.0)

            # ---- mask preload: [n, b, l] -> partitions (b%8)*16+n, group b//8
            mfs = []  # mf[g] [128 pairs, 256 l] f32
            for g in range(2):
                mi = consts.tile([128, LD], I32, tag=f"mi{g}")
                # partitions ordered (b_in_group, n); src iterates (b, n, l)
                src = msk_ap.rearrange("n (g b) l -> g b n l", g=2)[g]
                nc.sync.dma_start(out=mi, in_=src)
                mf = consts.tile([128, LD], F32, tag=f"mf{g}")
                nc.vector.tensor_copy(mf, mi)
                mfs.append(mf)
            # PE-transpose masks -> maskT[c][g] [128 l-in-chunk, 128 pairs]
            maskT = [[None, None], [None, None]]
            for g in range(2):
                for c in range(2):
                    pst = ps_misc.tile([128, 128], F32, tag="misc")
                    nc.tensor.transpose(pst, mfs[g][:, c * 128:(c + 1) * 128], ident)
                    mt = consts.tile([128, 128], F32, tag=f"mt{c}{g}")
                    nc.vector.tensor_copy(mt, pst)
                    maskT[c][g] = mt

            stage = consts.tile([32, BL * NWAY], F32)  # maxsim vectors, col j=b*16+n

            for bl in range(BL):
                g, pgrp = bl // 8, (bl % 8) * 16

                # ---- q normalize + transpose (tiny)
                qn = qpool.tile([LQ, D], F32, tag="qn")
                nc.sync.dma_start(out=qn, in_=q_ap[bl])
                qsq = qpool.tile([LQ, D], F32, tag="qsq")
                qss = small.tile([LQ, 1], F32, tag="qss")
                nc.scalar.activation(qsq, qn, AF.Square, accum_out=qss)
                qnrm = small.tile([LQ, 1], F32, tag="qnrm")
                nc.scalar.activation(qnrm, qss, AF.Sqrt)
                qri = small.tile([LQ, 1], F32, tag="qri")
                nc.vector.reciprocal(qri, qnrm)
                qns = qpool.tile([LQ, D], F32, tag="qns")
                nc.vector.tensor_scalar_mul(qns, qn, qri)
                ps_qt = ps_misc.tile([D, LQ], F32, tag="misc")
                nc.tensor.transpose(ps_qt, qns, ident[:LQ, :LQ])
                qT = qpool.tile([D, LQ], F32, tag="qT")
                nc.vector.tensor_copy(qT, ps_qt)

                ssq = small.tile([128, NWAY], F32, tag="ssq")
                rtiles = []
                for n in range(NWAY):
                    # ---- load doc[n, bl] as [p, c, d], l = c*128 + p
                    A = apool.tile([128, 2, D], F32, tag="A")
                    nc.sync.dma_start(
                        out=A,
                        in_=doc_ap[n, bl].rearrange("(c p) d -> p c d", p=128),
                    )
                    # ---- mask (per-partition scalar per chunk)
                    Am = apool.tile([128, 2, D], F32, tag="Am")
                    for c in range(2):
                        nc.gpsimd.tensor_scalar_mul(
                            Am[:, c, :], A[:, c, :],
                            maskT[c][g][:, pgrp + n:pgrp + n + 1],
                        )
                    # ---- transpose both chunks into one PSUM tile [128d, 256l]
                    pdt = ps_dt.tile([D, LD], F32, tag="pdt")
                    for c in range(2):
                        nc.tensor.transpose(
                            pdt[:, c * 128:(c + 1) * 128], Am[:, c, :], ident
                        )
                    R = rpool.tile([D, LD], F32, tag="R")
                    nc.scalar.activation(R, pdt, AF.Copy)
                    # ---- sumsq over l per feature d (ACT square + accum)
                    sq = scratch.tile([D, LD], F32, tag="sq")
                    nc.scalar.activation(sq, R, AF.Square,
                                         accum_out=ssq[:, n:n + 1])
                    rtiles.append(R)

                # ---- batched rsqrt for all 16 n of this b
                nrm = small.tile([128, NWAY], F32, tag="nrm")
                nc.scalar.activation(nrm, ssq, AF.Sqrt)
                rinv = small.tile([128, NWAY], F32, tag="rinv")
                nc.vector.reciprocal(rinv, nrm)

                for n in range(NWAY):
                    qTs = qpool.tile([D, LQ], F32, tag="qTs")
                    nc.vector.tensor_scalar_mul(qTs, qT, rinv[:, n:n + 1])
                    psim = ps_sim.tile([LQ, LD], F32, tag="psim")
                    nc.tensor.matmul(psim, lhsT=qTs, rhs=rtiles[n],
                                     start=True, stop=True)
                    j = bl * NWAY + n
                    nc.vector.reduce_max(
                        stage[:, j:j + 1], psim, axis=mybir.AxisListType.X
                    )

            # ---- scores[1, 256] = ones32.T @ stage ; reshape to [16b, 16n]
            ps_sc = ps_misc.tile([1, BL * NWAY], F32, tag="misc")
            nc.tensor.matmul(ps_sc, lhsT=ones32, rhs=stage, start=True, stop=True)
            sc_row = small.tile([1, BL * NWAY], F32, tag="scrow")
            nc.vector.tensor_copy(sc_row, ps_sc)
            dsc = dram.tile([BL, NWAY], F32, tag="dsc")
            nc.sync.dma_start(out=dsc.rearrange("b n -> (b n)")[None, :], in_=sc_row)
            sc = small.tile([BL, NWAY], F32, tag="sc")
            nc.sync.dma_start(out=sc, in_=dsc)

            # ---- softmax / KL / focal tail on [16, 16]
            mrow = small.tile([BL, 1], F32, tag="mrow")
            nc.vector.reduce_max(mrow, sc, axis=mybir.AxisListType.X)
            xs = small.tile([BL, NWAY], F32, tag="xs")
            nc.vector.tensor_scalar(xs, sc, mrow, None, op0=ALU.subtract)
            ex = small.tile([BL, NWAY], F32, tag="ex")
            srow = small.tile([BL, 1], F32, tag="srow")
            nc.scalar.activation(ex, xs, AF.Exp, accum_out=srow)
            lgs = small.tile([BL, 1], F32, tag="lgs")
            nc.scalar.activation(lgs, srow, AF.Ln)
            logp = small.tile([BL, NWAY], F32, tag="logp")
            nc.vector.tensor_scalar(logp, xs, lgs, None, op0=ALU.subtract)
            rs = small.tile([BL, 1], F32, tag="rs")
            nc.vector.reciprocal(rs, srow)
            p = small.tile([BL, NWAY], F32, tag="p")
            nc.vector.tensor_scalar_mul(p, ex, rs)

            labt = small.tile([BL, NWAY], F32, tag="labt")
            nc.sync.dma_start(out=labt, in_=lab_ap[:, 0:NWAY])
            expt = small.tile([BL, NWAY], F32, tag="expt")
            nc.scalar.activation(expt, labt, AF.Exp)
            tml = small.tile([BL, NWAY], F32, tag="tml")
            nc.vector.tensor_tensor(tml, labt, logp, op=ALU.subtract)
            kl = small.tile([BL, NWAY], F32, tag="kl")
            nc.vector.tensor_tensor(kl, expt, tml, op=ALU.mult)
            p2 = small.tile([BL, NWAY], F32, tag="p2")
            nc.vector.tensor_tensor(p2, p, p, op=ALU.mult)
            p4 = small.tile([BL, NWAY], F32, tag="p4")
            nc.vector.tensor_tensor(p4, p2, p2, op=ALU.mult)
            p5 = small.tile([BL, NWAY], F32, tag="p5")
            nc.vector.tensor_tensor(p5, p4, p, op=ALU.mult)
            lv = small.tile([BL, NWAY], F32, tag="lv")
            nc.vector.tensor_tensor(lv, kl, p5, op=ALU.mult)
            rsum = small.tile([BL, 1], F32, tag="rsum")
            nc.vector.reduce_sum(rsum, lv, axis=mybir.AxisListType.X)
            ps_tot = ps_misc.tile([1, 1], F32, tag="misc")
            nc.tensor.matmul(ps_tot, lhsT=ones16, rhs=rsum, start=True, stop=True)
            ot = small.tile([1, 1], F32, tag="ot")
            nc.vector.tensor_copy(ot, ps_tot)
            nc.sync.dma_start(out=out_ap, in_=ot)

    nc.finalize()
    return nc


def _get_nc():
    global _nc_cache
    if _nc_cache is None:
        _nc_cache = _build_nc()
    return _nc_cache


def run(inputs, trace=False):
    q = np.ascontiguousarray(np.asarray(inputs["query_reps"], dtype=np.float32))
    doc = np.ascontiguousarray(np.asarray(inputs["doc_reps"], dtype=np.float32))
    msk = np.ascontiguousarray(np.asarray(inputs["doc_masks"], dtype=np.int32))
    lab = np.ascontiguousarray(np.asarray(inputs["labels"], dtype=np.float32))

    in_maps = []
    for k in range(NCORES):
        b0 = k * BL
        in_maps.append({
            "q": np.ascontiguousarray(q[b0:b0 + BL]),
            "doc": np.ascontiguousarray(doc[:, b0:b0 + BL]),
            "msk": np.ascontiguousarray(msk[:, b0:b0 + BL]),
            "lab": np.ascontiguousarray(lab[b0:b0 + BL]),
        })

    nc = _get_nc()
    res = bass_utils.run_bass_kernel_spmd(
        nc, in_maps, core_ids=list(range(NCORES)), trace=trace
    )
    total = np.float64(0.0)
    for r in res.results:
        total += np.float64(r["out"][0, 0])
    loss = np.float32(total / (B * NWAY))
    return np.array(loss, dtype=np.float32), res


def kernel(**inputs) -> np.ndarray:
    out, _ = run(inputs, trace=False)
    return out
